# revision 22
# baseline (speedup 1.0000x reference)
"""Trainium2 Bass kernel for nn_Druggability_DistillModel (gnn_message_passing).

Strategy (8 NeuronCores, data-parallel over B x 4-way sequence shards):
  - core c handles batch b=c//4, tokens [s*512, (s+1)*512), s=c%4; per-core
    inputs are token-rotated so the shard is always rows 0:512.
  - Graph attention is dense-E: softmax_k(q.k/16 + edge) * v ==
    (exp(q.hK^T/16) * E) @ h @ (Wv Wlo) / rowsum; E[j,t] is host-built from
    the 65-entry edge-bias table (duplicate neighbors merge by summing).
    The value aggregation contracts RAW h (single-head attention commutes
    with the value projection); Wv@Wlo is applied once to the normalized
    aggregate in the tail.
  - LN1 is host-folded: the device receives h^T (bf16+fp8) and token-major
    h (fp8); no LN1 chain and no PE transposes at all.
  - All 256-contraction matmuls run fp8e4 DoubleRow; the E-multiply runs on
    the otherwise-idle GPSIMD engine in fp8.
  - ACT table discipline: exp_and_others for everything (gelu-for-gates via
    tanh approx, sigmoid via tanh) except one Sqrt visit (LN2) and one Gelu
    visit (FFN); reciprocals use the fast custom-DVE approximation.
"""
import sys

sys.path.insert(0, "/opt/trn_rl_repo")

import numpy as np
import ml_dtypes

B, L, D, H, DH, K, DE, CLIP = 2, 2048, 256, 8, 32, 36, 64, 32
NCORES, SPB, SH = 8, 4, 512
NT = L // 128
BF16 = ml_dtypes.bfloat16
FP8 = ml_dtypes.float8_e4m3

_CACHE: dict = {}


def _gelu_np(x):
    try:
        from scipy.special import erf
        e = erf(x / np.sqrt(2.0))
    except Exception:
        import math as _m
        e = np.vectorize(_m.erf)(x / np.sqrt(2.0))
    return x * 0.5 * (1.0 + e)


def _w_tiles(w, cin_chunks, dt=BF16):
    """[din, dout] -> [128, cin_chunks, dout] with din = c*128+p."""
    din, dout = w.shape
    assert din == cin_chunks * 128
    return np.ascontiguousarray(
        w.reshape(cin_chunks, 128, dout).transpose(1, 0, 2)
    ).astype(dt)


def _build(taps=()):
    import concourse.bass as bass
    import concourse.tile as tile
    from concourse import bacc, mybir

    f32, bf = mybir.dt.float32, mybir.dt.bfloat16
    f8 = mybir.dt.float8e4
    AF = mybir.ActivationFunctionType
    ALU = mybir.AluOpType
    DR = mybir.MatmulPerfMode.DoubleRow
    GS = 0.850683  # gelu(x) ~ x*(0.5 + 0.5*tanh(GS*x)); 0.5 folded into W2

    nc = bacc.Bacc("TRN2", target_bir_lowering=False, debug=False)

    ht_d = nc.dram_tensor("ht", [128, 2, L], bf, kind="ExternalInput")
    ht8_d = nc.dram_tensor("ht8", [128, 2, L], f8, kind="ExternalInput")
    htok_d = nc.dram_tensor("htok", [128, NT, D], f8, kind="ExternalInput")
    el_d = nc.dram_tensor("el", [128, NT, SH], f8, kind="ExternalInput")
    wq_d = nc.dram_tensor("wq", [128, 2, D], f8, kind="ExternalInput")
    wk_d = nc.dram_tensor("wk", [128, 2, D], f8, kind="ExternalInput")
    wvlo_d = nc.dram_tensor("wvlo", [128, 2, D], f8, kind="ExternalInput")
    wqkv_d = nc.dram_tensor("wqkv", [128, 2, 3 * D], f8, kind="ExternalInput")
    wg1_d = nc.dram_tensor("wg1", [128, 4, D], f8, kind="ExternalInput")
    wg2_d = nc.dram_tensor("wg2", [128, 2, D], f8, kind="ExternalInput")
    wgo_d = nc.dram_tensor("wgo", [128, 2, D], bf, kind="ExternalInput")
    wf1_d = nc.dram_tensor("wf1", [128, 2, D], f8, kind="ExternalInput")
    wf2_d = nc.dram_tensor("wf2", [128, 2, 2], bf, kind="ExternalInput")
    wff1_d = nc.dram_tensor("wff1", [128, 2, 4 * D], f8, kind="ExternalInput")
    wff2_d = nc.dram_tensor("wff2", [128, 8, D], f8, kind="ExternalInput")
    selc_d = nc.dram_tensor("selc", [8, 2, 128], bf, kind="ExternalInput")
    pm_d = nc.dram_tensor("pm", [2, 1], bf, kind="ExternalInput")
    maskh_d = nc.dram_tensor("maskh", [128, 2, 8], bf, kind="ExternalInput")
    out_d = nc.dram_tensor("out", [128, 2, SH], f32, kind="ExternalOutput")
    tap_tiles = {}

    with tile.TileContext(nc) as tc:
        with (
            tc.tile_pool(name="const", bufs=1) as const,
            tc.tile_pool(name="persist", bufs=1) as pers,
            tc.tile_pool(name="stm", bufs=4) as stm,
            tc.tile_pool(name="stmq", bufs=8) as stmq,
            tc.tile_pool(name="stmf", bufs=8) as stmf,
            tc.tile_pool(name="utp", bufs=4) as utp,
            tc.tile_pool(name="psA", bufs=2, space="PSUM") as psA,
            tc.tile_pool(name="psB", bufs=1, space="PSUM") as psB,
            tc.tile_pool(name="psacc", bufs=4, space="PSUM") as psacc,
            tc.tile_pool(name="pssml", bufs=1, space="PSUM") as pssml,
        ):
            ones_cb = const.tile([128, 1], bf)
            nc.vector.memset(ones_cb[:], 1.0)
            ones_c8 = const.tile([128, 2, 128], f8)
            nc.vector.memset(ones_c8[:], 1.0)
            ones_rb = const.tile([1, 128], bf)
            nc.vector.memset(ones_rb[:], 1.0)
            eps5 = const.tile([1, 1], f32)
            nc.vector.memset(eps5[:], 1e-5)
            selc = const.tile([8, 2, 128], bf)
            nc.sync.dma_start(selc[:], selc_d[:])
            maskh = const.tile([128, 2, 8], bf)
            nc.sync.dma_start(maskh[:], maskh_d[:])
            pm = const.tile([2, 1], bf)
            nc.sync.dma_start(pm[:], pm_d[:])

            def wload(dram, shape, dt):
                t = const.tile(list(shape), dt, tag=dram.name)
                nc.sync.dma_start(t[:], dram[:])
                return t

            hT = pers.tile([128, 2, L], bf)
            hT8 = pers.tile([128, 2, L], f8)
            htok = pers.tile([128, NT, D], f8)
            el_all = pers.tile([128, NT, SH], f8)
            # whole-tensor transfers (contiguous >=2KB per-partition lines),
            # ordered by first use: sweep inputs, then tail-only tensors
            wqkv = wload(wqkv_d, (128, 2, 3 * D), f8)
            nc.sync.dma_start(hT8[:], ht8_d[:])
            wk = wload(wk_d, (128, 2, D), f8)
            wq = wload(wq_d, (128, 2, D), f8)
            wf1 = wload(wf1_d, (128, 2, D), f8)
            wf2 = wload(wf2_d, (128, 2, 2), bf)
            nc.sync.dma_start(el_all[:], el_d[:])
            nc.sync.dma_start(htok[:], htok_d[:])
            nc.sync.dma_start(hT[:], ht_d[:])
            wvlo = wload(wvlo_d, (128, 2, D), f8)
            wg1 = wload(wg1_d, (128, 4, D), f8)
            wg2 = wload(wg2_d, (128, 2, D), f8)
            wgo = wload(wgo_d, (128, 2, D), bf)
            wff1 = wload(wff1_d, (128, 2, 4 * D), f8)
            wff2 = wload(wff2_d, (128, 8, D), f8)

            hKT = pers.tile([128, 2, L], f8)
            qT = pers.tile([128, 2, SH], f8)
            kg8 = pers.tile([128, NT, D], f8)
            vg8 = pers.tile([128, NT, D + 1], f8)
            nc.vector.memset(vg8[:, :, D:D + 1], 1.0)
            qg_b = pers.tile([128, 2, SH], bf)
            tap_tiles["qT"] = qT
            tap_tiles["hKT"] = hKT

            kv_ps = [psacc.tile([128, 257], f32, tag="acc", name=f"kv{g}")
                     for g in range(2)]
            agg_ps = [psacc.tile([128, SH], f32, tag="acc", name=f"agg{g}")
                      for g in range(2)]
            den_ps = pssml.tile([128, SH], f32, tag="accs")

            # ---------- emission helpers ----------
            def emit_kv(n):
                if n % 2 == 1:
                    return
                for g in range(2):
                    nc.tensor.matmul(
                        kv_ps[g][:], kg8[:, n:n + 2, g * 128:(g + 1) * 128],
                        vg8[:, n:n + 2, :], start=(n == 0), stop=(n == NT - 2),
                        perf_mode=DR)

            ut_tiles = {}

            def emit_attn_acc(jc):
                if jc % 2 == 1:
                    return
                ut = ut_tiles.pop(jc)
                nc.tensor.matmul(den_ps[:], ones_c8[:], ut[:],
                                 start=(jc == 0), stop=(jc == NT - 2),
                                 perf_mode=DR)
                for g in range(2):
                    nc.tensor.matmul(agg_ps[g][:],
                                     htok[:, jc:jc + 2, g * 128:(g + 1) * 128],
                                     ut[:], start=(jc == 0),
                                     stop=(jc == NT - 2), perf_mode=DR)

            def emit_prework(n):
                js = slice(n * 128, (n + 1) * 128)
                pq = psA.tile([128, 512], f32, tag="mm")
                nc.tensor.matmul(pq[:], hT8[:, :, js], wqkv[:, :, D:3 * D],
                                 start=True, stop=True, perf_mode=DR)
                # kg = elu(x)+1 = min(exp(x),1) + relu(x)
                te = stmq.tile([128, D], bf, tag="tmpq")
                nc.scalar.activation(te[:], pq[:, 0:D], AF.Exp)
                m1 = stmq.tile([128, D], bf, tag="tmpq")
                nc.vector.tensor_scalar_min(m1[:], te[:], 1.0)
                nc.vector.scalar_tensor_tensor(
                    kg8[:, n, :], pq[:, 0:D], 0.0, m1[:],
                    op0=ALU.max, op1=ALU.add)
                nc.vector.tensor_copy(vg8[:, n, 0:D], pq[:, D:2 * D])
                if n >= 2:
                    emit_kv(n - 2)

            def emit_attn(jc):
                pl = psB.tile([128, 512], f32, tag="mm")
                nc.tensor.matmul(pl[:], hKT[:, :, jc * 128:(jc + 1) * 128],
                                 qT[:], start=True, stop=True, perf_mode=DR)
                ux = stmf.tile([128, 512], f8, tag="tmpf")
                nc.scalar.activation(ux[:], pl[:], AF.Exp, scale=1.0 / 16.0)
                if jc % 2 == 0:
                    utpair = utp.tile([128, 2, 512], f8, tag="ut")
                    ut_tiles[jc] = utpair
                else:
                    utpair = ut_tiles[jc - 1]
                nc.gpsimd.tensor_mul(utpair[:, jc % 2, :], ux[:],
                                     el_all[:, jc, :])
                if jc >= 2:
                    emit_attn_acc(jc - 2)

            # ---------- prologue: group 0 prework + shard-local chains ------
            for n in range(4):
                emit_prework(n)
            for g in range(2):
                pk = psA.tile([128, 512], f32, tag="mm")
                nc.tensor.matmul(pk[:], wk[:, :, g * 128:(g + 1) * 128],
                                 hT8[:, :, 0:SH], start=True, stop=True,
                                 perf_mode=DR)
                nc.scalar.copy(hKT[:, g, 0:SH], pk[:])
            for g in range(2):
                pq2 = psA.tile([128, 512], f32, tag="mm")
                nc.tensor.matmul(pq2[:], wq[:, :, g * 128:(g + 1) * 128],
                                 hT8[:, :, 0:SH], start=True, stop=True,
                                 perf_mode=DR)
                nc.scalar.copy(qT[:, g, :], pq2[:])
            # qg (linear-attn queries)
            for g in range(2):
                pq3 = psA.tile([128, 512], f32, tag="mm")
                nc.tensor.matmul(pq3[:], wqkv[:, :, g * 128:(g + 1) * 128],
                                 hT8[:, :, 0:SH], start=True, stop=True,
                                 perf_mode=DR)
                teb = stmf.tile([128, 512], bf, tag="tmpf")
                nc.scalar.activation(teb[:], pq3[:], AF.Exp)
                m1b = stmf.tile([128, 512], bf, tag="tmpf")
                nc.vector.tensor_scalar_min(m1b[:], teb[:], 1.0)
                nc.vector.scalar_tensor_tensor(
                    qg_b[:, g, :], pq3[:], 0.0, m1b[:],
                    op0=ALU.max, op1=ALU.add)
            # wf chain: f1 = gelu_tanh(wf1.T h)  (0.5 folded into wf2)
            f1T = pers.tile([128, 2, SH], f8)
            for g in range(2):
                pf = psA.tile([128, 512], f32, tag="mm")
                nc.tensor.matmul(pf[:], wf1[:, :, g * 128:(g + 1) * 128],
                                 hT8[:, :, 0:SH], start=True, stop=True,
                                 perf_mode=DR)
                tt = stmf.tile([128, 512], bf, tag="tmpf")
                nc.scalar.activation(tt[:], pf[:], AF.Tanh, scale=GS)
                nc.vector.scalar_tensor_tensor(f1T[:, g, :], tt[:], 1.0,
                                               pf[:], op0=ALU.add, op1=ALU.mult)
            wf_ps = psA.tile([2, SH], f32, tag="mm", name="wfp")
            for c in range(2):
                nc.tensor.matmul(wf_ps[:], wf2[:, c, :], f1T[:, c, :],
                                 start=(c == 0), stop=(c == 1))
            wf_sb = stm.tile([2, SH], bf, tag="wf_sb")
            nc.scalar.copy(wf_sb[:], wf_ps[:])
            d01_ps = psA.tile([1, SH], f32, tag="mm", name="d01")
            nc.tensor.matmul(d01_ps[:], pm[:], wf_sb[:], start=True, stop=True)
            th = pers.tile([1, SH], bf)
            nc.scalar.activation(th[:], d01_ps[:], AF.Tanh, scale=0.5)

            # ---------- groups 1-3: prework(g) interleaved with attn(g-1) ---
            for qgrp in range(1, 4):
                for i in range(4):
                    n = qgrp * 4 + i
                    emit_attn(n - 4)
                    emit_prework(n)
                jsg = slice(qgrp * 512, (qgrp + 1) * 512)
                for g in range(2):
                    pk = psA.tile([128, 512], f32, tag="mm")
                    nc.tensor.matmul(pk[:], wk[:, :, g * 128:(g + 1) * 128],
                                     hT8[:, :, jsg], start=True, stop=True,
                                     perf_mode=DR)
                    nc.scalar.copy(hKT[:, g, jsg], pk[:])
            for jc in range(12, 16):
                emit_attn(jc)
            emit_kv(NT - 2)
            emit_kv(NT - 1)
            emit_attn_acc(NT - 2)
            emit_attn_acc(NT - 1)

            # ---------- tail ----------
            # den reciprocal (fast approx) + broadcast
            den_f = stm.tile([1, SH], f32, tag="den_f")
            nc.vector.reciprocal_approx_fast(den_f[:], den_ps[0:1, :])
            den_r = stm.tile([1, SH], bf, tag="den_r")
            nc.scalar.copy(den_r[:], den_f[:])
            rbp = psB.tile([128, 512], f32, tag="mm", name="rbp")
            nc.tensor.matmul(rbp[:], ones_rb[:], den_r[:], start=True, stop=True)
            rb_sb = stmf.tile([128, 512], f32, tag="tmpf", name="rb_sb")
            nc.scalar.copy(rb_sb[:], rbp[:])
            # normalized raw aggregate (fp8) then project by WvWlo
            aggraw8 = pers.tile([128, 2, SH], f8)
            for g in range(2):
                nc.vector.tensor_mul(aggraw8[:, g, :], agg_ps[g][:], rb_sb[:])
            # kv block-diagonal + z (before psacc banks get recycled)
            kvb = pers.tile([128, 2, D], bf)
            nc.vector.memset(kvb[:], 0.0)
            for h in range(H):
                g, po = h // 4, (h * DH) % 128
                nc.scalar.copy(kvb[po:po + DH, g, h * DH:(h + 1) * DH],
                               kv_ps[g][po:po + DH, h * DH:(h + 1) * DH])
            tap_tiles["kvb"] = kvb
            ksel = pers.tile([128, 2, 8], bf)
            for g in range(2):
                nc.vector.tensor_scalar(ksel[:, g, :], maskh[:, g, :],
                                        kv_ps[g][:, D:D + 1], None,
                                        op0=ALU.mult)
            zden_ps = pssml.tile([8, SH], f32, tag="accs", name="zden")
            for g in range(2):
                nc.tensor.matmul(zden_ps[:], ksel[:, g, :], qg_b[:, g, :],
                                 start=(g == 0), stop=(g == 1))
            zr0 = stm.tile([8, SH], f32, tag="zr0")
            nc.vector.tensor_scalar_add(zr0[:], zden_ps[:], 1e-6)
            zr = stm.tile([8, SH], f32, tag="zr")
            nc.vector.reciprocal_approx_fast(zr[:], zr0[:])
            zr_b = stm.tile([8, SH], bf, tag="zr_b")
            nc.vector.tensor_copy(zr_b[:], zr[:])
            qgzT = pers.tile([128, 2, SH], bf)
            for g in range(2):
                pzb = psA.tile([128, 512], f32, tag="mm")
                nc.tensor.matmul(pzb[:], selc[:, g, :], zr_b[:],
                                 start=True, stop=True)
                zrs = stmf.tile([128, 512], bf, tag="tmpf")
                nc.scalar.copy(zrs[:], pzb[:])
                nc.vector.tensor_mul(qgzT[:, g, :], qg_b[:, g, :], zrs[:])
            tap_tiles["qgzT"] = qgzT

            aggloT = pers.tile([128, 2, SH], bf)
            agglo8 = pers.tile([128, 2, SH], f8)
            for g in range(2):
                pa = psA.tile([128, 512], f32, tag="mm")
                nc.tensor.matmul(pa[:], wvlo[:, :, g * 128:(g + 1) * 128],
                                 aggraw8[:], start=True, stop=True, perf_mode=DR)
                nc.scalar.copy(agglo8[:, g, :], pa[:])
                nc.vector.tensor_copy(aggloT[:, g, :], pa[:])
            tap_tiles["aggloT"] = aggloT

            # gate chain (tanh forms, all in the exp table set)
            g1T = pers.tile([128, 2, SH], f8)
            for g in range(2):
                pg = psA.tile([128, 512], f32, tag="mm")
                gsl = slice(g * 128, (g + 1) * 128)
                nc.tensor.matmul(pg[:], wg1[:, 0:2, gsl], hT8[:, :, 0:SH],
                                 start=True, stop=False, perf_mode=DR)
                nc.tensor.matmul(pg[:], wg1[:, 2:4, gsl], agglo8[:],
                                 start=False, stop=True, perf_mode=DR)
                tt = stmf.tile([128, 512], bf, tag="tmpf")
                nc.scalar.activation(tt[:], pg[:], AF.Tanh, scale=GS)
                nc.vector.scalar_tensor_tensor(g1T[:, g, :], tt[:], 1.0,
                                               pg[:], op0=ALU.add, op1=ALU.mult)
            tgT = pers.tile([128, 2, SH], bf)
            for g in range(2):
                pg2 = psA.tile([128, 512], f32, tag="mm")
                nc.tensor.matmul(pg2[:], wg2[:, :, g * 128:(g + 1) * 128],
                                 g1T[:], start=True, stop=True, perf_mode=DR)
                nc.scalar.activation(tgT[:, g, :], pg2[:], AF.Tanh, scale=0.5)
            # h_local = h + 0.5*(agglo + tg*agglo)
            h_localT = pers.tile([128, 2, SH], f32)
            for g in range(2):
                ga = stmf.tile([128, 512], f32, tag="tmpf")
                nc.gpsimd.tensor_mul(ga[:], tgT[:, g, :], aggloT[:, g, :])
                gs = stmf.tile([128, 512], f32, tag="tmpf")
                nc.gpsimd.tensor_add(gs[:], ga[:], aggloT[:, g, :])
                nc.vector.scalar_tensor_tensor(h_localT[:, g, :], gs[:], 0.5,
                                               hT[:, g, 0:SH],
                                               op0=ALU.mult, op1=ALU.add)
            tap_tiles["h_localT"] = h_localT

            # linear attention y and h_global
            yT = pers.tile([128, 2, SH], bf)
            for g in range(2):
                py = psA.tile([128, 512], f32, tag="mm")
                for c in range(2):
                    nc.tensor.matmul(py[:], kvb[:, c, g * 128:(g + 1) * 128],
                                     qgzT[:, c, :], start=(c == 0), stop=(c == 1))
                nc.vector.tensor_copy(yT[:, g, :], py[:])
            h_globalT = pers.tile([128, 2, SH], f32)
            for g in range(2):
                pgo = psA.tile([128, 512], f32, tag="mm")
                for c in range(2):
                    nc.tensor.matmul(pgo[:], wgo[:, c, g * 128:(g + 1) * 128],
                                     yT[:, c, :], start=(c == 0), stop=(c == 1))
                nc.vector.tensor_add(h_globalT[:, g, :], hT[:, g, 0:SH], pgo[:])
            tap_tiles["h_globalT"] = h_globalT

            # xo = hg + sigmoid(d01)*(hl-hg);  sigmoid = 0.5*(1+th)
            thb_ps = psB.tile([128, 512], f32, tag="mm", name="thb")
            nc.tensor.matmul(thb_ps[:], ones_rb[:], th[:], start=True, stop=True)
            xoT = pers.tile([128, 2, SH], f32)
            xo_b = pers.tile([128, 2, SH], bf)
            for g in range(2):
                dlg = stmf.tile([128, 512], f32, tag="tmpf")
                nc.vector.tensor_sub(dlg[:], h_localT[:, g, :], h_globalT[:, g, :])
                u = stmf.tile([128, 512], f32, tag="tmpf")
                nc.vector.scalar_tensor_tensor(u[:], dlg[:], 1.0, thb_ps[:],
                                               op0=ALU.mult, op1=ALU.mult)
                nc.vector.tensor_add(u[:], u[:], dlg[:])
                nc.vector.scalar_tensor_tensor(xoT[:, g, :], u[:], 0.5,
                                               h_globalT[:, g, :],
                                               op0=ALU.mult, op1=ALU.add)
                nc.scalar.copy(xo_b[:, g, :], xoT[:, g, :])
            tap_tiles["xoT"] = xoT

            # LN2 (g2 folded into Wff1; b2 == 0)
            sum_ps = pssml.tile([1, SH], f32, tag="accs", name="s1")
            for c in range(2):
                nc.tensor.matmul(sum_ps[:], ones_cb[:], xo_b[:, c, :],
                                 start=(c == 0), stop=(c == 1))
            xsq = pers.tile([128, 2, SH], bf)
            for c in range(2):
                nc.scalar.activation(xsq[:, c, :], xoT[:, c, :], AF.Square)
            ssq_ps = psA.tile([1, SH], f32, tag="mm", name="ssq")
            for c in range(2):
                nc.tensor.matmul(ssq_ps[:], ones_cb[:], xsq[:, c, :],
                                 start=(c == 0), stop=(c == 1))
            mean = stm.tile([1, SH], f32, tag="mean")
            nc.scalar.mul(mean[:], sum_ps[:], 1.0 / D)
            var = stm.tile([1, SH], f32, tag="var")
            nc.vector.scalar_tensor_tensor(var[:], mean[:], -1.0, mean[:],
                                           op0=ALU.mult, op1=ALU.mult)
            nc.vector.scalar_tensor_tensor(var[:], ssq_ps[:], 1.0 / D, var[:],
                                           op0=ALU.mult, op1=ALU.add)
            sd2 = stm.tile([1, SH], f32, tag="sd2")
            nc.scalar.activation(sd2[:], var[:], AF.Sqrt, bias=eps5[0:1, 0:1])
            rstd = stm.tile([1, SH], f32, tag="rstd")
            nc.vector.reciprocal_approx_fast(rstd[:], sd2[:])
            rstd_b = stm.tile([1, SH], bf, tag="rstd_b")
            nc.vector.tensor_copy(rstd_b[:], rstd[:])
            nmr = stm.tile([1, SH], bf, tag="nmr")
            with nc.allow_low_precision("nmr"):
                nc.vector.scalar_tensor_tensor(nmr[:], mean[:], -1.0, rstd[:],
                                               op0=ALU.mult, op1=ALU.mult)
            rb2 = psA.tile([128, 512], f32, tag="mm", name="rb2")
            nc.tensor.matmul(rb2[:], ones_rb[:], rstd_b[:], start=True, stop=True)
            nm2 = psB.tile([128, 512], f32, tag="mm", name="nm2")
            nc.tensor.matmul(nm2[:], ones_rb[:], nmr[:], start=True, stop=True)
            xnT = pers.tile([128, 2, SH], f8)
            for c in range(2):
                t1 = stmf.tile([128, 512], f32, tag="tmpf")
                nc.vector.tensor_mul(t1[:], xoT[:, c, :], rb2[:])
                nc.vector.tensor_add(xnT[:, c, :], t1[:], nm2[:])
            tap_tiles["xnT"] = xnT

            # FFN (exact Gelu table)
            ff1T = pers.tile([128, 8, SH], f8)
            for g8 in range(8):
                pff = psA.tile([128, 512], f32, tag="mm")
                nc.tensor.matmul(pff[:], wff1[:, :, g8 * 128:(g8 + 1) * 128],
                                 xnT[:], start=True, stop=True, perf_mode=DR)
                nc.scalar.activation(ff1T[:, g8, :], pff[:], AF.Gelu)
            outT = pers.tile([128, 2, SH], f32)
            tap_tiles["outT"] = outT
            for g in range(2):
                pf2 = psA.tile([128, 512], f32, tag="mm")
                gsl = slice(g * 128, (g + 1) * 128)
                for k2 in range(4):
                    nc.tensor.matmul(pf2[:], wff2[:, 2 * k2:2 * k2 + 2, gsl],
                                     ff1T[:, 2 * k2:2 * k2 + 2, :],
                                     start=(k2 == 0), stop=(k2 == 3),
                                     perf_mode=DR)
                nc.vector.tensor_add(outT[:, g, :], xoT[:, g, :], pf2[:])
                nc.sync.dma_start(out_d[:, g, :], outT[:, g, :])

            for name in taps:
                t = tap_tiles[name]
                td = nc.dram_tensor(f"tap_{name}", list(t.shape),
                                    t.dtype, kind="ExternalOutput")
                nc.sync.dma_start(td[:], t[:])

    nc.compile()
    return nc


def _host_prep(inputs):
    """Host-side preprocessing shared by all cores + per-core arrays."""
    x = np.asarray(inputs["x"], np.float32)
    mask = np.asarray(inputs["mask"])
    nbr_idx = np.asarray(inputs["nbr_idx"]).astype(np.int64)
    nbr_mask = np.asarray(inputs["nbr_mask"])
    rel_pos = np.asarray(inputs["rel_pos"]).astype(np.int64)

    if not (np.all(mask == 1)):
        raise NotImplementedError("kernel assumes mask == ones (spec fill)")
    for k in ("blo", "bg1", "bg2", "bf1", "bf2", "bff1", "bff2", "b2"):
        if not np.allclose(np.asarray(inputs[k]), 0.0):
            raise NotImplementedError(f"kernel assumes bias {k} == 0")

    # LN1 on host -> h
    g1 = np.asarray(inputs["g1"], np.float32)
    b1 = np.asarray(inputs["b1"], np.float32)
    m = x.mean(-1, keepdims=True)
    v = x.var(-1, keepdims=True)
    h = (x - m) / np.sqrt(v + 1e-5) * g1 + b1          # [B, L, D]

    # edge-bias table -> dense E (exp-ed, duplicates summed)
    Erel = np.asarray(inputs["Erel"], np.float32)
    We1 = np.asarray(inputs["We1"], np.float32)
    be1 = np.asarray(inputs["be1"], np.float32)
    We2 = np.asarray(inputs["We2"], np.float32)
    be2 = np.asarray(inputs["be2"], np.float32)
    tab = (_gelu_np(Erel @ We1 + be1) @ We2 + be2)[:, 0]  # [65]

    rel = np.clip(rel_pos, -CLIP, CLIP) + CLIP
    ev = np.exp(tab[rel]) * (nbr_mask != 0)            # [B, L, K]
    ET = np.zeros((B, L, L), np.float32)
    for b in range(B):
        t_idx = np.repeat(np.arange(L), K)
        np.add.at(ET[b], (nbr_idx[b].ravel(), t_idx), ev[b].ravel())

    Wvlo = np.asarray(inputs["Wv"], np.float32) @ np.asarray(inputs["Wlo"], np.float32)
    g2 = np.asarray(inputs["g2"], np.float32)

    selc = np.zeros((8, 2, 128), np.float32)
    maskh = np.zeros((128, 2, 8), np.float32)
    for c in range(2):
        for p in range(128):
            hh = (c * 128 + p) // DH
            selc[hh, c, p] = 1.0
            maskh[p, c, hh] = 1.0

    shared = {
        "wq": _w_tiles(np.asarray(inputs["Wq"], np.float32), 2, FP8),
        "wk": _w_tiles(np.asarray(inputs["Wk"], np.float32), 2, FP8),
        "wvlo": _w_tiles(Wvlo, 2, FP8),
        "wqkv": _w_tiles(np.asarray(inputs["Wqkv"], np.float32), 2, FP8),
        "wg1": _w_tiles(np.asarray(inputs["Wg1"], np.float32), 4, FP8),
        "wg2": _w_tiles(0.5 * np.asarray(inputs["Wg2"], np.float32), 2, FP8),
        "wgo": _w_tiles(np.asarray(inputs["Wgo"], np.float32), 2, BF16),
        "wf1": _w_tiles(np.asarray(inputs["Wf1"], np.float32), 2, FP8),
        "wf2": _w_tiles(0.5 * np.asarray(inputs["Wf2"], np.float32), 2, BF16),
        "wff1": _w_tiles(g2[:, None] * np.asarray(inputs["Wff1"], np.float32),
                         2, FP8),
        "wff2": _w_tiles(np.asarray(inputs["Wff2"], np.float32), 8, FP8),
        "selc": selc.astype(BF16),
        "pm": np.array([[1.0], [-1.0]], BF16),
        "maskh": maskh.astype(BF16),
    }

    per_core = []
    for c in range(NCORES):
        b, s = c // SPB, c % SPB
        s0 = s * SH
        hp = np.roll(h[b], -s0, axis=0)                 # [L, D]
        hTp = np.ascontiguousarray(
            hp.T.reshape(2, 128, L).transpose(1, 0, 2))  # [128, 2, L]
        htokp = np.ascontiguousarray(
            hp.reshape(NT, 128, D).transpose(1, 0, 2))   # [128, NT, D]
        elp = np.roll(ET[b][:, s0:s0 + SH], -s0, axis=0)
        elp = np.ascontiguousarray(
            elp.reshape(NT, 128, SH).transpose(1, 0, 2))  # [128, NT, SH]
        per_core.append({
            "ht": hTp.astype(BF16),
            "ht8": hTp.astype(FP8),
            "htok": htokp.astype(FP8),
            "el": elp.astype(FP8),
        })
    return shared, per_core


def kernel(**inputs) -> np.ndarray:
    import concourse.bass_utils as bu

    if "nc" not in _CACHE:
        _CACHE["nc"] = _build()
    nc = _CACHE["nc"]

    shared, per_core = _host_prep(inputs)
    in_maps = [{**shared, **pc} for pc in per_core]
    res = bu.run_bass_kernel_spmd(nc, in_maps, core_ids=list(range(NCORES)))
    out = np.zeros((B, L, D), np.float32)
    for c in range(NCORES):
        b, s = c // SPB, c % SPB
        o = res.results[c]["out"]                       # [128, 2, SH]
        out[b, s * SH:(s + 1) * SH] = o.transpose(2, 1, 0).reshape(SH, D)
    return out


# revision 24
# speedup vs baseline: 1.0286x; 1.0286x over previous
"""Trainium2 Bass kernel for nn_Druggability_DistillModel (gnn_message_passing).

Strategy (8 NeuronCores, data-parallel over B x 4-way sequence shards):
  - core c handles batch b=c//4, tokens [s*512, (s+1)*512), s=c%4; per-core
    inputs are token-rotated so the shard is always rows 0:512.
  - Graph attention is dense-E: softmax_k(q.k/16 + edge) * v ==
    (exp(q.hK^T/16) * E) @ h @ (Wv Wlo) / rowsum; E[j,t] is host-built from
    the 65-entry edge-bias table (duplicate neighbors merge by summing).
    The value aggregation contracts RAW h (single-head attention commutes
    with the value projection); Wv@Wlo is applied once to the normalized
    aggregate in the tail.
  - LN1 is host-folded: the device receives h^T (bf16+fp8) and token-major
    h (fp8); no LN1 chain and no PE transposes at all.
  - All 256-contraction matmuls run fp8e4 DoubleRow; the E-multiply runs on
    the otherwise-idle GPSIMD engine in fp8.
  - ACT table discipline: exp_and_others for everything (gelu-for-gates via
    tanh approx, sigmoid via tanh) except one Sqrt visit (LN2) and one Gelu
    visit (FFN); reciprocals use the fast custom-DVE approximation.
"""
import sys

sys.path.insert(0, "/opt/trn_rl_repo")

import numpy as np
import ml_dtypes

B, L, D, H, DH, K, DE, CLIP = 2, 2048, 256, 8, 32, 36, 64, 32
NCORES, SPB, SH = 8, 4, 512
NT = L // 128
BF16 = ml_dtypes.bfloat16
FP8 = ml_dtypes.float8_e4m3

_CACHE: dict = {}


def _gelu_np(x):
    try:
        from scipy.special import erf
        e = erf(x / np.sqrt(2.0))
    except Exception:
        import math as _m
        e = np.vectorize(_m.erf)(x / np.sqrt(2.0))
    return x * 0.5 * (1.0 + e)


def _w_tiles(w, cin_chunks, dt=BF16):
    """[din, dout] -> [128, cin_chunks, dout] with din = c*128+p."""
    din, dout = w.shape
    assert din == cin_chunks * 128
    return np.ascontiguousarray(
        w.reshape(cin_chunks, 128, dout).transpose(1, 0, 2)
    ).astype(dt)


def _build(taps=()):
    import concourse.bass as bass
    import concourse.tile as tile
    from concourse import bacc, mybir

    f32, bf = mybir.dt.float32, mybir.dt.bfloat16
    f8 = mybir.dt.float8e4
    AF = mybir.ActivationFunctionType
    ALU = mybir.AluOpType
    DR = mybir.MatmulPerfMode.DoubleRow
    GS = 0.850683  # gelu(x) ~ x*(0.5 + 0.5*tanh(GS*x)); 0.5 folded into W2

    nc = bacc.Bacc("TRN2", target_bir_lowering=False, debug=False)

    ht_d = nc.dram_tensor("ht", [128, 2, L], bf, kind="ExternalInput")
    ht8_d = nc.dram_tensor("ht8", [128, 2, L], f8, kind="ExternalInput")
    htok_d = nc.dram_tensor("htok", [128, NT, D], f8, kind="ExternalInput")
    el_d = nc.dram_tensor("el", [L, SH], f8, kind="ExternalInput")
    wq_d = nc.dram_tensor("wq", [128, 2, D], f8, kind="ExternalInput")
    wk_d = nc.dram_tensor("wk", [128, 2, D], f8, kind="ExternalInput")
    wvlo_d = nc.dram_tensor("wvlo", [128, 2, D], f8, kind="ExternalInput")
    wqkv_d = nc.dram_tensor("wqkv", [128, 2, 3 * D], f8, kind="ExternalInput")
    wg1_d = nc.dram_tensor("wg1", [128, 4, D], f8, kind="ExternalInput")
    wg2_d = nc.dram_tensor("wg2", [128, 2, D], f8, kind="ExternalInput")
    wgo_d = nc.dram_tensor("wgo", [128, 2, D], bf, kind="ExternalInput")
    wf1_d = nc.dram_tensor("wf1", [128, 2, D], f8, kind="ExternalInput")
    wf2_d = nc.dram_tensor("wf2", [128, 2, 2], bf, kind="ExternalInput")
    wff1_d = nc.dram_tensor("wff1", [128, 2, 4 * D], f8, kind="ExternalInput")
    wff2_d = nc.dram_tensor("wff2", [128, 8, D], f8, kind="ExternalInput")
    selc_d = nc.dram_tensor("selc", [8, 2, 128], bf, kind="ExternalInput")
    pm_d = nc.dram_tensor("pm", [2, 1], bf, kind="ExternalInput")
    maskh_d = nc.dram_tensor("maskh", [128, 2, 8], bf, kind="ExternalInput")
    out_d = nc.dram_tensor("out", [128, 2, SH], f32, kind="ExternalOutput")
    tap_tiles = {}

    with tile.TileContext(nc) as tc:
        with (
            tc.tile_pool(name="const", bufs=1) as const,
            tc.tile_pool(name="persist", bufs=1) as pers,
            tc.tile_pool(name="stm", bufs=4) as stm,
            tc.tile_pool(name="stmq", bufs=8) as stmq,
            tc.tile_pool(name="stmf", bufs=8) as stmf,
            tc.tile_pool(name="utp", bufs=4) as utp,
            tc.tile_pool(name="psA", bufs=2, space="PSUM") as psA,
            tc.tile_pool(name="psB", bufs=1, space="PSUM") as psB,
            tc.tile_pool(name="psacc", bufs=4, space="PSUM") as psacc,
            tc.tile_pool(name="pssml", bufs=1, space="PSUM") as pssml,
        ):
            ones_cb = const.tile([128, 1], bf)
            nc.vector.memset(ones_cb[:], 1.0)
            ones_c8 = const.tile([128, 2, 128], f8)
            nc.vector.memset(ones_c8[:], 1.0)
            ones_rb = const.tile([1, 128], bf)
            nc.vector.memset(ones_rb[:], 1.0)
            eps5 = const.tile([1, 1], f32)
            nc.vector.memset(eps5[:], 1e-5)
            selc = const.tile([8, 2, 128], bf)
            nc.sync.dma_start(selc[:], selc_d[:])
            maskh = const.tile([128, 2, 8], bf)
            nc.sync.dma_start(maskh[:], maskh_d[:])
            pm = const.tile([2, 1], bf)
            nc.sync.dma_start(pm[:], pm_d[:])

            def wload(dram, shape, dt):
                t = const.tile(list(shape), dt, tag=dram.name)
                nc.sync.dma_start(t[:], dram[:])
                return t

            hT = pers.tile([128, 2, L], bf)
            hT8 = pers.tile([128, 2, L], f8)
            htok = pers.tile([128, NT, D], f8)
            # first group + early weights first
            nc.sync.dma_start(hT8[:, :, 0:SH], ht8_d[:, :, 0:SH])
            wqkv = wload(wqkv_d, (128, 2, 3 * D), f8)
            nc.sync.dma_start(htok[:, 0:4, :], htok_d[:, 0:4, :])
            wk = wload(wk_d, (128, 2, D), f8)
            wq = wload(wq_d, (128, 2, D), f8)
            wf1 = wload(wf1_d, (128, 2, D), f8)
            wf2 = wload(wf2_d, (128, 2, 2), bf)
            nc.sync.dma_start(hT[:, :, 0:SH], ht_d[:, :, 0:SH])
            for qg_ in range(1, 4):
                sl = slice(qg_ * SH, (qg_ + 1) * SH)
                nc.sync.dma_start(hT8[:, :, sl], ht8_d[:, :, sl])
                nc.sync.dma_start(htok[:, qg_ * 4:(qg_ + 1) * 4, :],
                                  htok_d[:, qg_ * 4:(qg_ + 1) * 4, :])
                nc.sync.dma_start(hT[:, :, sl], ht_d[:, :, sl])
            el_r = el_d.rearrange("(n p) t -> p n t", p=128)
            el_all = pers.tile([128, NT, SH], f8)
            for qg_ in range(4):
                nc.sync.dma_start(el_all[:, qg_ * 4:(qg_ + 1) * 4, :],
                                  el_r[:, qg_ * 4:(qg_ + 1) * 4, :])
            wvlo = wload(wvlo_d, (128, 2, D), f8)
            wg1 = wload(wg1_d, (128, 4, D), f8)
            wg2 = wload(wg2_d, (128, 2, D), f8)
            wgo = wload(wgo_d, (128, 2, D), bf)
            wff1 = wload(wff1_d, (128, 2, 4 * D), f8)
            wff2 = wload(wff2_d, (128, 8, D), f8)

            hKT = pers.tile([128, 2, L], f8)
            qT = pers.tile([128, 2, SH], f8)
            kg8 = pers.tile([128, NT, D], f8)
            vg8 = pers.tile([128, NT, D + 1], f8)
            nc.vector.memset(vg8[:, :, D:D + 1], 1.0)
            qg_b = pers.tile([128, 2, SH], bf)
            tap_tiles["qT"] = qT
            tap_tiles["hKT"] = hKT

            kv_ps = [psacc.tile([128, 257], f32, tag="acc", name=f"kv{g}")
                     for g in range(2)]
            agg_ps = [psacc.tile([128, SH], f32, tag="acc", name=f"agg{g}")
                      for g in range(2)]
            den_ps = pssml.tile([128, SH], f32, tag="accs")

            # ---------- emission helpers ----------
            def emit_kv(n):
                if n % 2 == 1:
                    return
                for g in range(2):
                    nc.tensor.matmul(
                        kv_ps[g][:], kg8[:, n:n + 2, g * 128:(g + 1) * 128],
                        vg8[:, n:n + 2, :], start=(n == 0), stop=(n == NT - 2),
                        perf_mode=DR)

            ut_tiles = {}

            def emit_attn_acc(jc):
                if jc % 2 == 1:
                    return
                ut = ut_tiles.pop(jc)
                nc.tensor.matmul(den_ps[:], ones_c8[:], ut[:],
                                 start=(jc == 0), stop=(jc == NT - 2),
                                 perf_mode=DR)
                for g in range(2):
                    nc.tensor.matmul(agg_ps[g][:],
                                     htok[:, jc:jc + 2, g * 128:(g + 1) * 128],
                                     ut[:], start=(jc == 0),
                                     stop=(jc == NT - 2), perf_mode=DR)

            def emit_prework(n):
                js = slice(n * 128, (n + 1) * 128)
                pq = psA.tile([128, 512], f32, tag="mm")
                nc.tensor.matmul(pq[:], hT8[:, :, js], wqkv[:, :, D:3 * D],
                                 start=True, stop=True, perf_mode=DR)
                # kg = elu(x)+1 = min(exp(x),1) + relu(x)
                te = stmq.tile([128, D], bf, tag="tmpq")
                nc.scalar.activation(te[:], pq[:, 0:D], AF.Exp)
                m1 = stmq.tile([128, D], bf, tag="tmpq")
                nc.vector.tensor_scalar_min(m1[:], te[:], 1.0)
                nc.vector.scalar_tensor_tensor(
                    kg8[:, n, :], pq[:, 0:D], 0.0, m1[:],
                    op0=ALU.max, op1=ALU.add)
                nc.vector.tensor_copy(vg8[:, n, 0:D], pq[:, D:2 * D])
                if n >= 2:
                    emit_kv(n - 2)

            def emit_attn(jc):
                pl = psB.tile([128, 512], f32, tag="mm")
                nc.tensor.matmul(pl[:], hKT[:, :, jc * 128:(jc + 1) * 128],
                                 qT[:], start=True, stop=True, perf_mode=DR)
                ux = stmf.tile([128, 512], f8, tag="tmpf")
                nc.scalar.activation(ux[:], pl[:], AF.Exp, scale=1.0 / 16.0)
                if jc % 2 == 0:
                    utpair = utp.tile([128, 2, 512], f8, tag="ut")
                    ut_tiles[jc] = utpair
                else:
                    utpair = ut_tiles[jc - 1]
                nc.gpsimd.tensor_mul(utpair[:, jc % 2, :], ux[:],
                                     el_all[:, jc, :])
                if jc >= 2:
                    emit_attn_acc(jc - 2)

            # ---------- prologue: group 0 prework + shard-local chains ------
            for n in range(4):
                emit_prework(n)
            for g in range(2):
                pk = psA.tile([128, 512], f32, tag="mm")
                nc.tensor.matmul(pk[:], wk[:, :, g * 128:(g + 1) * 128],
                                 hT8[:, :, 0:SH], start=True, stop=True,
                                 perf_mode=DR)
                nc.scalar.copy(hKT[:, g, 0:SH], pk[:])
            for g in range(2):
                pq2 = psA.tile([128, 512], f32, tag="mm")
                nc.tensor.matmul(pq2[:], wq[:, :, g * 128:(g + 1) * 128],
                                 hT8[:, :, 0:SH], start=True, stop=True,
                                 perf_mode=DR)
                nc.scalar.copy(qT[:, g, :], pq2[:])
            # qg (linear-attn queries)
            for g in range(2):
                pq3 = psA.tile([128, 512], f32, tag="mm")
                nc.tensor.matmul(pq3[:], wqkv[:, :, g * 128:(g + 1) * 128],
                                 hT8[:, :, 0:SH], start=True, stop=True,
                                 perf_mode=DR)
                teb = stmf.tile([128, 512], bf, tag="tmpf")
                nc.scalar.activation(teb[:], pq3[:], AF.Exp)
                m1b = stmf.tile([128, 512], bf, tag="tmpf")
                nc.vector.tensor_scalar_min(m1b[:], teb[:], 1.0)
                nc.vector.scalar_tensor_tensor(
                    qg_b[:, g, :], pq3[:], 0.0, m1b[:],
                    op0=ALU.max, op1=ALU.add)
            # wf chain: f1 = gelu_tanh(wf1.T h)  (0.5 folded into wf2)
            f1T = pers.tile([128, 2, SH], f8)
            for g in range(2):
                pf = psA.tile([128, 512], f32, tag="mm")
                nc.tensor.matmul(pf[:], wf1[:, :, g * 128:(g + 1) * 128],
                                 hT8[:, :, 0:SH], start=True, stop=True,
                                 perf_mode=DR)
                tt = stmf.tile([128, 512], bf, tag="tmpf")
                nc.scalar.activation(tt[:], pf[:], AF.Tanh, scale=GS)
                nc.vector.scalar_tensor_tensor(f1T[:, g, :], tt[:], 1.0,
                                               pf[:], op0=ALU.add, op1=ALU.mult)
            wf_ps = psA.tile([2, SH], f32, tag="mm", name="wfp")
            for c in range(2):
                nc.tensor.matmul(wf_ps[:], wf2[:, c, :], f1T[:, c, :],
                                 start=(c == 0), stop=(c == 1))
            wf_sb = stm.tile([2, SH], bf, tag="wf_sb")
            nc.scalar.copy(wf_sb[:], wf_ps[:])
            d01_ps = psA.tile([1, SH], f32, tag="mm", name="d01")
            nc.tensor.matmul(d01_ps[:], pm[:], wf_sb[:], start=True, stop=True)
            th = pers.tile([1, SH], bf)
            nc.scalar.activation(th[:], d01_ps[:], AF.Tanh, scale=0.5)

            # ---------- groups 1-3: prework(g) interleaved with attn(g-1) ---
            for qgrp in range(1, 4):
                for i in range(4):
                    n = qgrp * 4 + i
                    emit_attn(n - 4)
                    emit_prework(n)
                jsg = slice(qgrp * 512, (qgrp + 1) * 512)
                for g in range(2):
                    pk = psA.tile([128, 512], f32, tag="mm")
                    nc.tensor.matmul(pk[:], wk[:, :, g * 128:(g + 1) * 128],
                                     hT8[:, :, jsg], start=True, stop=True,
                                     perf_mode=DR)
                    nc.scalar.copy(hKT[:, g, jsg], pk[:])
            for jc in range(12, 16):
                emit_attn(jc)
            emit_kv(NT - 2)
            emit_kv(NT - 1)
            emit_attn_acc(NT - 2)
            emit_attn_acc(NT - 1)

            # ---------- tail ----------
            # den reciprocal (fast approx) + broadcast
            den_f = stm.tile([1, SH], f32, tag="den_f")
            nc.vector.reciprocal_approx_fast(den_f[:], den_ps[0:1, :])
            den_r = stm.tile([1, SH], bf, tag="den_r")
            nc.scalar.copy(den_r[:], den_f[:])
            rbp = psB.tile([128, 512], f32, tag="mm", name="rbp")
            nc.tensor.matmul(rbp[:], ones_rb[:], den_r[:], start=True, stop=True)
            rb_sb = stmf.tile([128, 512], f32, tag="tmpf", name="rb_sb")
            nc.scalar.copy(rb_sb[:], rbp[:])
            # normalized raw aggregate (fp8) then project by WvWlo
            aggraw8 = pers.tile([128, 2, SH], f8)
            for g in range(2):
                nc.vector.tensor_mul(aggraw8[:, g, :], agg_ps[g][:], rb_sb[:])
            # kv block-diagonal + z (before psacc banks get recycled)
            kvb = pers.tile([128, 2, D], bf)
            nc.vector.memset(kvb[:], 0.0)
            for h in range(H):
                g, po = h // 4, (h * DH) % 128
                nc.scalar.copy(kvb[po:po + DH, g, h * DH:(h + 1) * DH],
                               kv_ps[g][po:po + DH, h * DH:(h + 1) * DH])
            tap_tiles["kvb"] = kvb
            ksel = pers.tile([128, 2, 8], bf)
            for g in range(2):
                nc.vector.tensor_scalar(ksel[:, g, :], maskh[:, g, :],
                                        kv_ps[g][:, D:D + 1], None,
                                        op0=ALU.mult)
            zden_ps = pssml.tile([8, SH], f32, tag="accs", name="zden")
            for g in range(2):
                nc.tensor.matmul(zden_ps[:], ksel[:, g, :], qg_b[:, g, :],
                                 start=(g == 0), stop=(g == 1))
            zr0 = stm.tile([8, SH], f32, tag="zr0")
            nc.vector.tensor_scalar_add(zr0[:], zden_ps[:], 1e-6)
            zr = stm.tile([8, SH], f32, tag="zr")
            nc.vector.reciprocal_approx_fast(zr[:], zr0[:])
            zr_b = stm.tile([8, SH], bf, tag="zr_b")
            nc.vector.tensor_copy(zr_b[:], zr[:])
            qgzT = pers.tile([128, 2, SH], bf)
            for g in range(2):
                pzb = psA.tile([128, 512], f32, tag="mm")
                nc.tensor.matmul(pzb[:], selc[:, g, :], zr_b[:],
                                 start=True, stop=True)
                zrs = stmf.tile([128, 512], bf, tag="tmpf")
                nc.scalar.copy(zrs[:], pzb[:])
                nc.vector.tensor_mul(qgzT[:, g, :], qg_b[:, g, :], zrs[:])
            tap_tiles["qgzT"] = qgzT

            aggloT = pers.tile([128, 2, SH], bf)
            agglo8 = pers.tile([128, 2, SH], f8)
            for g in range(2):
                pa = psA.tile([128, 512], f32, tag="mm")
                nc.tensor.matmul(pa[:], wvlo[:, :, g * 128:(g + 1) * 128],
                                 aggraw8[:], start=True, stop=True, perf_mode=DR)
                nc.scalar.copy(agglo8[:, g, :], pa[:])
                nc.vector.tensor_copy(aggloT[:, g, :], pa[:])
            tap_tiles["aggloT"] = aggloT

            # gate chain (tanh forms, all in the exp table set)
            g1T = pers.tile([128, 2, SH], f8)
            for g in range(2):
                pg = psA.tile([128, 512], f32, tag="mm")
                gsl = slice(g * 128, (g + 1) * 128)
                nc.tensor.matmul(pg[:], wg1[:, 0:2, gsl], hT8[:, :, 0:SH],
                                 start=True, stop=False, perf_mode=DR)
                nc.tensor.matmul(pg[:], wg1[:, 2:4, gsl], agglo8[:],
                                 start=False, stop=True, perf_mode=DR)
                tt = stmf.tile([128, 512], bf, tag="tmpf")
                nc.scalar.activation(tt[:], pg[:], AF.Tanh, scale=GS)
                nc.vector.scalar_tensor_tensor(g1T[:, g, :], tt[:], 1.0,
                                               pg[:], op0=ALU.add, op1=ALU.mult)
            tgT = pers.tile([128, 2, SH], bf)
            for g in range(2):
                pg2 = psA.tile([128, 512], f32, tag="mm")
                nc.tensor.matmul(pg2[:], wg2[:, :, g * 128:(g + 1) * 128],
                                 g1T[:], start=True, stop=True, perf_mode=DR)
                nc.scalar.activation(tgT[:, g, :], pg2[:], AF.Tanh, scale=0.5)
            # h_local = h + 0.5*(agglo + tg*agglo)
            h_localT = pers.tile([128, 2, SH], f32)
            for g in range(2):
                ga = stmf.tile([128, 512], f32, tag="tmpf")
                nc.gpsimd.tensor_mul(ga[:], tgT[:, g, :], aggloT[:, g, :])
                gs = stmf.tile([128, 512], f32, tag="tmpf")
                nc.gpsimd.tensor_add(gs[:], ga[:], aggloT[:, g, :])
                nc.vector.scalar_tensor_tensor(h_localT[:, g, :], gs[:], 0.5,
                                               hT[:, g, 0:SH],
                                               op0=ALU.mult, op1=ALU.add)
            tap_tiles["h_localT"] = h_localT

            # linear attention y and h_global
            yT = pers.tile([128, 2, SH], bf)
            for g in range(2):
                py = psA.tile([128, 512], f32, tag="mm")
                for c in range(2):
                    nc.tensor.matmul(py[:], kvb[:, c, g * 128:(g + 1) * 128],
                                     qgzT[:, c, :], start=(c == 0), stop=(c == 1))
                nc.vector.tensor_copy(yT[:, g, :], py[:])
            h_globalT = pers.tile([128, 2, SH], f32)
            for g in range(2):
                pgo = psA.tile([128, 512], f32, tag="mm")
                for c in range(2):
                    nc.tensor.matmul(pgo[:], wgo[:, c, g * 128:(g + 1) * 128],
                                     yT[:, c, :], start=(c == 0), stop=(c == 1))
                nc.vector.tensor_add(h_globalT[:, g, :], hT[:, g, 0:SH], pgo[:])
            tap_tiles["h_globalT"] = h_globalT

            # xo = hg + sigmoid(d01)*(hl-hg);  sigmoid = 0.5*(1+th)
            thb_ps = psB.tile([128, 512], f32, tag="mm", name="thb")
            nc.tensor.matmul(thb_ps[:], ones_rb[:], th[:], start=True, stop=True)
            xoT = pers.tile([128, 2, SH], f32)
            xo_b = pers.tile([128, 2, SH], bf)
            for g in range(2):
                dlg = stmf.tile([128, 512], f32, tag="tmpf")
                nc.vector.tensor_sub(dlg[:], h_localT[:, g, :], h_globalT[:, g, :])
                u = stmf.tile([128, 512], f32, tag="tmpf")
                nc.vector.scalar_tensor_tensor(u[:], dlg[:], 1.0, thb_ps[:],
                                               op0=ALU.mult, op1=ALU.mult)
                nc.vector.tensor_add(u[:], u[:], dlg[:])
                nc.vector.scalar_tensor_tensor(xoT[:, g, :], u[:], 0.5,
                                               h_globalT[:, g, :],
                                               op0=ALU.mult, op1=ALU.add)
                nc.scalar.copy(xo_b[:, g, :], xoT[:, g, :])
            tap_tiles["xoT"] = xoT

            # LN2 (g2 folded into Wff1; b2 == 0)
            sum_ps = pssml.tile([1, SH], f32, tag="accs", name="s1")
            for c in range(2):
                nc.tensor.matmul(sum_ps[:], ones_cb[:], xo_b[:, c, :],
                                 start=(c == 0), stop=(c == 1))
            xsq = pers.tile([128, 2, SH], bf)
            for c in range(2):
                nc.vector.tensor_mul(xsq[:, c, :], xo_b[:, c, :], xo_b[:, c, :])
            ssq_ps = psA.tile([1, SH], f32, tag="mm", name="ssq")
            for c in range(2):
                nc.tensor.matmul(ssq_ps[:], ones_cb[:], xsq[:, c, :],
                                 start=(c == 0), stop=(c == 1))
            mean = stm.tile([1, SH], f32, tag="mean")
            nc.scalar.mul(mean[:], sum_ps[:], 1.0 / D)
            var = stm.tile([1, SH], f32, tag="var")
            nc.vector.scalar_tensor_tensor(var[:], mean[:], -1.0, mean[:],
                                           op0=ALU.mult, op1=ALU.mult)
            nc.vector.scalar_tensor_tensor(var[:], ssq_ps[:], 1.0 / D, var[:],
                                           op0=ALU.mult, op1=ALU.add)
            sd2 = stm.tile([1, SH], f32, tag="sd2")
            nc.scalar.activation(sd2[:], var[:], AF.Sqrt, bias=eps5[0:1, 0:1])
            dumg = stm.tile([1, 1], f32, tag="dumg")
            nc.scalar.activation(dumg[:], eps5[:], AF.Gelu)
            rstd = stm.tile([1, SH], f32, tag="rstd")
            nc.vector.reciprocal_approx_fast(rstd[:], sd2[:])
            rstd_b = stm.tile([1, SH], bf, tag="rstd_b")
            nc.vector.tensor_copy(rstd_b[:], rstd[:])
            nmr = stm.tile([1, SH], bf, tag="nmr")
            with nc.allow_low_precision("nmr"):
                nc.vector.scalar_tensor_tensor(nmr[:], mean[:], -1.0, rstd[:],
                                               op0=ALU.mult, op1=ALU.mult)
            rb2 = psA.tile([128, 512], f32, tag="mm", name="rb2")
            nc.tensor.matmul(rb2[:], ones_rb[:], rstd_b[:], start=True, stop=True)
            nm2 = psB.tile([128, 512], f32, tag="mm", name="nm2")
            nc.tensor.matmul(nm2[:], ones_rb[:], nmr[:], start=True, stop=True)
            xnT = pers.tile([128, 2, SH], f8)
            for c in range(2):
                t1 = stmf.tile([128, 512], f32, tag="tmpf")
                nc.vector.tensor_mul(t1[:], xoT[:, c, :], rb2[:])
                nc.vector.tensor_add(xnT[:, c, :], t1[:], nm2[:])
            tap_tiles["xnT"] = xnT

            # FFN (exact Gelu table)
            ff1T = pers.tile([128, 8, SH], f8)
            for g8 in range(8):
                pff = psA.tile([128, 512], f32, tag="mm")
                nc.tensor.matmul(pff[:], wff1[:, :, g8 * 128:(g8 + 1) * 128],
                                 xnT[:], start=True, stop=True, perf_mode=DR)
                nc.scalar.activation(ff1T[:, g8, :], pff[:], AF.Gelu)
            outT = pers.tile([128, 2, SH], f32)
            tap_tiles["outT"] = outT
            for g in range(2):
                pf2 = psA.tile([128, 512], f32, tag="mm")
                gsl = slice(g * 128, (g + 1) * 128)
                for k2 in range(4):
                    nc.tensor.matmul(pf2[:], wff2[:, 2 * k2:2 * k2 + 2, gsl],
                                     ff1T[:, 2 * k2:2 * k2 + 2, :],
                                     start=(k2 == 0), stop=(k2 == 3),
                                     perf_mode=DR)
                nc.vector.tensor_add(outT[:, g, :], xoT[:, g, :], pf2[:])
                nc.sync.dma_start(out_d[:, g, :], outT[:, g, :])

            for name in taps:
                t = tap_tiles[name]
                td = nc.dram_tensor(f"tap_{name}", list(t.shape),
                                    t.dtype, kind="ExternalOutput")
                nc.sync.dma_start(td[:], t[:])

    nc.compile()
    return nc


def _host_prep(inputs):
    """Host-side preprocessing shared by all cores + per-core arrays."""
    x = np.asarray(inputs["x"], np.float32)
    mask = np.asarray(inputs["mask"])
    nbr_idx = np.asarray(inputs["nbr_idx"]).astype(np.int64)
    nbr_mask = np.asarray(inputs["nbr_mask"])
    rel_pos = np.asarray(inputs["rel_pos"]).astype(np.int64)

    if not (np.all(mask == 1)):
        raise NotImplementedError("kernel assumes mask == ones (spec fill)")
    for k in ("blo", "bg1", "bg2", "bf1", "bf2", "bff1", "bff2", "b2"):
        if not np.allclose(np.asarray(inputs[k]), 0.0):
            raise NotImplementedError(f"kernel assumes bias {k} == 0")

    # LN1 on host -> h
    g1 = np.asarray(inputs["g1"], np.float32)
    b1 = np.asarray(inputs["b1"], np.float32)
    m = x.mean(-1, keepdims=True)
    v = x.var(-1, keepdims=True)
    h = (x - m) / np.sqrt(v + 1e-5) * g1 + b1          # [B, L, D]

    # edge-bias table -> dense E (exp-ed, duplicates summed)
    Erel = np.asarray(inputs["Erel"], np.float32)
    We1 = np.asarray(inputs["We1"], np.float32)
    be1 = np.asarray(inputs["be1"], np.float32)
    We2 = np.asarray(inputs["We2"], np.float32)
    be2 = np.asarray(inputs["be2"], np.float32)
    tab = (_gelu_np(Erel @ We1 + be1) @ We2 + be2)[:, 0]  # [65]

    rel = np.clip(rel_pos, -CLIP, CLIP) + CLIP
    ev = np.exp(tab[rel]) * (nbr_mask != 0)            # [B, L, K]
    ET = np.zeros((B, L, L), np.float32)
    for b in range(B):
        t_idx = np.repeat(np.arange(L), K)
        np.add.at(ET[b], (nbr_idx[b].ravel(), t_idx), ev[b].ravel())

    Wvlo = np.asarray(inputs["Wv"], np.float32) @ np.asarray(inputs["Wlo"], np.float32)
    g2 = np.asarray(inputs["g2"], np.float32)

    selc = np.zeros((8, 2, 128), np.float32)
    maskh = np.zeros((128, 2, 8), np.float32)
    for c in range(2):
        for p in range(128):
            hh = (c * 128 + p) // DH
            selc[hh, c, p] = 1.0
            maskh[p, c, hh] = 1.0

    shared = {
        "wq": _w_tiles(np.asarray(inputs["Wq"], np.float32), 2, FP8),
        "wk": _w_tiles(np.asarray(inputs["Wk"], np.float32), 2, FP8),
        "wvlo": _w_tiles(Wvlo, 2, FP8),
        "wqkv": _w_tiles(np.asarray(inputs["Wqkv"], np.float32), 2, FP8),
        "wg1": _w_tiles(np.asarray(inputs["Wg1"], np.float32), 4, FP8),
        "wg2": _w_tiles(0.5 * np.asarray(inputs["Wg2"], np.float32), 2, FP8),
        "wgo": _w_tiles(np.asarray(inputs["Wgo"], np.float32), 2, BF16),
        "wf1": _w_tiles(np.asarray(inputs["Wf1"], np.float32), 2, FP8),
        "wf2": _w_tiles(0.5 * np.asarray(inputs["Wf2"], np.float32), 2, BF16),
        "wff1": _w_tiles(g2[:, None] * np.asarray(inputs["Wff1"], np.float32),
                         2, FP8),
        "wff2": _w_tiles(np.asarray(inputs["Wff2"], np.float32), 8, FP8),
        "selc": selc.astype(BF16),
        "pm": np.array([[1.0], [-1.0]], BF16),
        "maskh": maskh.astype(BF16),
    }

    per_core = []
    for c in range(NCORES):
        b, s = c // SPB, c % SPB
        s0 = s * SH
        hp = np.roll(h[b], -s0, axis=0)                 # [L, D]
        hTp = np.ascontiguousarray(
            hp.T.reshape(2, 128, L).transpose(1, 0, 2))  # [128, 2, L]
        htokp = np.ascontiguousarray(
            hp.reshape(NT, 128, D).transpose(1, 0, 2))   # [128, NT, D]
        elp = np.roll(ET[b][:, s0:s0 + SH], -s0, axis=0)
        per_core.append({
            "ht": hTp.astype(BF16),
            "ht8": hTp.astype(FP8),
            "htok": htokp.astype(FP8),
            "el": np.ascontiguousarray(elp).astype(FP8),
        })
    return shared, per_core


def kernel(**inputs) -> np.ndarray:
    import concourse.bass_utils as bu

    if "nc" not in _CACHE:
        _CACHE["nc"] = _build()
    nc = _CACHE["nc"]

    shared, per_core = _host_prep(inputs)
    in_maps = [{**shared, **pc} for pc in per_core]
    res = bu.run_bass_kernel_spmd(nc, in_maps, core_ids=list(range(NCORES)))
    out = np.zeros((B, L, D), np.float32)
    for c in range(NCORES):
        b, s = c // SPB, c % SPB
        o = res.results[c]["out"]                       # [128, 2, SH]
        out[b, s * SH:(s + 1) * SH] = o.transpose(2, 1, 0).reshape(SH, D)
    return out
